# revision 27
# baseline (speedup 1.0000x reference)
"""Trainium2 Bass kernel for a transformer encoder layer (nn_Encoder).

x:[2,2048,1024] f32. 8 NeuronCores, data-parallel: core c handles batch
n=c//4, query rows qi=c%4 (512 tokens). K/V are recomputed per core for
the full batch (x4 redundancy) to avoid collectives, which are far too
slow (~300us for the 8.4MB all-reduce this would replace).
All matmuls run as float32r (full PE rate, ~1e-4 rel err).
"""
import os
import sys
from contextlib import ExitStack

for _p in ("/opt/trn_rl_repo", "/root/.axon_site/_ro/trn_rl_repo"):
    if os.path.isdir(_p) and _p not in sys.path:
        sys.path.insert(0, _p)

import numpy as np
import concourse.bass as bass
import concourse.mybir as mybir
import concourse.tile as tile
from concourse import bacc
from concourse.bass_utils import run_bass_kernel_spmd
from concourse.masks import make_identity

F32 = mybir.dt.float32
F32R = mybir.dt.float32r
AF = mybir.ActivationFunctionType
ALU = mybir.AluOpType

D = 1024
H = 16
HD = 64
FF = 4096
L = 2048
NB = 2
P = 128
QT = 512          # query tokens per core
DC = D // P       # 8 chunks of the model dim
KT = L // P       # 16 key tiles
FC = FF // P      # 32 ff chunks
TT = QT // P      # 4 own-token tiles
NPAIR = H // 2    # 8 head pairs
EPS = 1e-5

_CACHED_NC = {}


def _build_nc(affine=True):
    nc = bacc.Bacc("TRN2", target_bir_lowering=False)

    din = {}

    def dparam(name, shape, dt=F32R):
        din[name] = nc.dram_tensor(name, shape, dt, kind="ExternalInput")
        return din[name]

    xT = dparam("xT", [D, L])              # x[n].T
    xTq = dparam("xTq", [D, QT])           # own-token columns of xT
    xq = dparam("xq", [QT, D], F32)        # own tokens, natural (residual)
    wq = dparam("wq", [NPAIR, DC, P, P])   # [pair, dc, dpart, cols]
    wk = dparam("wk", [NPAIR, DC, P, P])
    wv = dparam("wv", [2, DC, P, D // 2])  # [vcol-half, dc, dpart, 512]
    wo = dparam("wo", [DC, P, D])          # [hd-chunk, hd-part, ocols]
    w1 = dparam("w1", [FC, DC, P, P])      # [fc, dc, dpart, fcols]
    w2 = dparam("w2", [FC, P, D])          # [fc, ff-part, ocols]
    bq = dparam("bq", [P, NPAIR], F32)     # per-partition bias, by pair
    bk = dparam("bk", [P, NPAIR], F32)
    b1 = dparam("b1", [P, FC], F32)
    bvb = dparam("bvb", [P, D], F32)       # host-broadcast per-column params
    b2b = dparam("b2b", [P, D], F32)
    g1b = dparam("g1b", [P, D], F32)
    be1b = dparam("be1b", [P, D], F32)
    g2b = dparam("g2b", [P, D], F32)
    be2b = dparam("be2b", [P, D], F32)
    vones = dparam("vones", [P, KT], F32R)

    y = nc.dram_tensor("y", [QT, D], F32, kind="ExternalOutput")

    with tile.TileContext(nc) as tc:
        with tc.tile_pool(name="pers", bufs=1) as pers:
            ident = pers.tile([P, P], F32, tag="ident")
            make_identity(nc, ident[:])
            bq_t = pers.tile([P, NPAIR], F32, tag="bq")
            bk_t = pers.tile([P, NPAIR], F32, tag="bk")
            b1_t = pers.tile([P, FC], F32, tag="b1")
            eps_t = pers.tile([P, 1], F32, tag="eps")
            nc.gpsimd.memset(eps_t[:], EPS)
            zero_t = pers.tile([P, 1], F32, tag="zero")
            nc.gpsimd.memset(zero_t[:], 0.0)

            # ---- persistent blobs (tag-shared slots across phases) ----
            # blobA: xT (proj) -> ff1T (ffn);  both 64KB/partition
            # blobB: v_aug (proj+attn) -> wo (out-proj) -> hT (ffn)
            # tok1:  xTq (q-proj) -> outSB (attn out, transposed)
            # tok2:  qT (proj+attn) -> h (post-LN1, natural)
            xT_t = pers.tile([P, DC, L], F32R, tag="blobA")
            _att_es = ExitStack()
            vap = _att_es.enter_context(tc.tile_pool(name="vap", bufs=1))
            v_aug = vap.tile([P, KT, H * (HD + 1)], F32R, tag="vaug")
            ones_t = pers.tile([P, KT], F32R, tag="ones")
            xTq_t = pers.tile([P, DC, QT], F32R, tag="tok1")
            nc.sync.dma_start(xTq_t[:], xTq.rearrange("(c p) t -> p c t", p=P))
            nc.scalar.dma_start(bq_t[:], bq[:])
            nc.scalar.dma_start(bk_t[:], bk[:])
            nc.scalar.dma_start(b1_t[:], b1[:])

            # ================= Q projection (own tokens) ==============
            qT_t = pers.tile([P, NPAIR, QT], F32R, tag="tok2")
            with tc.tile_pool(name="qp", bufs=2) as qp, \
                 tc.tile_pool(name="qps", bufs=4, space="PSUM") as qps:
                for pr2 in range(NPAIR // 2):
                    wq_t = qp.tile([P, 2, DC, P], F32R, tag="w")
                    nc.scalar.dma_start(
                        wq_t[:], wq[2 * pr2:2 * pr2 + 2].rearrange("r c p m -> p r c m"))
                    for r in range(2):
                        pr = 2 * pr2 + r
                        ps = qps.tile([P, 512], F32, tag="mm")
                        for dc in range(DC):
                            nc.tensor.matmul(ps[:], wq_t[:, r, dc, :], xTq_t[:, dc, :],
                                             start=(dc == 0), stop=(dc == DC - 1))
                        nc.vector.tensor_scalar(qT_t[:, pr, :], ps[:],
                                                bq_t[:, pr:pr + 1], None, ALU.add)

            for dc in range(DC):
                nc.sync.dma_start(
                    xT_t[:, dc, :],
                    xT.rearrange("(c p) t -> p c t", p=P)[:, dc, :])
            nc.scalar.dma_start(ones_t[:], vones[:])
            nc.vector.tensor_copy(
                v_aug[:].rearrange("p t (h c) -> p t h c", c=HD + 1)[:, :, :, HD],
                ones_t[:, :, None].to_broadcast([P, KT, H]))

            # ================= V projection (dc-outer, streamed wv) ===
            with tc.tile_pool(name="vp", bufs=1) as vp, \
                 tc.tile_pool(name="vpw", bufs=2) as vpw, \
                 tc.tile_pool(name="vps", bufs=1, space="PSUM") as vps:
                bvb_t = vp.tile([P, D], F32, tag="bvb")
                nc.scalar.dma_start(bvb_t[:], bvb[:])
                wv_ts = [vpw.tile([P, DC, 512], F32R, tag="wvh", name=f"wvh{vh}")
                         for vh in range(2)]
                # interleave wv[0] per-dc with the xT chunks so V's first
                # matmuls aren't stuck behind the full 8.4MB xT stream
                for dc in range(DC):
                    nc.scalar.dma_start(wv_ts[0][:, dc, :], wv[0, dc])
                nc.scalar.dma_start(wv_ts[1][:], wv[1].rearrange("c p m -> p c m"))
                for vh in range(2):
                    wv_t = wv_ts[vh]
                    for pas in range(2):
                        ps8 = [vps.tile([P, 512], F32, tag=f"vmm{i}",
                                        name=f"vps_{vh}_{pas}_{i}") for i in range(8)]
                        for dc in range(DC):
                            for i in range(8):
                                tt = pas * 8 + i
                                nc.tensor.matmul(
                                    ps8[i], xT_t[:, dc, tt * P:(tt + 1) * P],
                                    wv_t[:, dc, :], start=(dc == 0), stop=(dc == DC - 1))
                        for i in range(8):
                            tt = pas * 8 + i
                            dst = v_aug[:, tt, :].rearrange(
                                "p (h c) -> p h c", c=HD + 1)[:, vh * 8:(vh + 1) * 8, 0:HD]
                            nc.vector.tensor_tensor(
                                dst, ps8[i].rearrange("p (h c) -> p h c", c=HD),
                                bvb_t[:, vh * 512:(vh + 1) * 512].rearrange(
                                    "p (h c) -> p h c", c=HD),
                                ALU.add)

            # ========== K projection fused with attention, per pair ====
            # kT for a pair stays in SBUF (no DRAM bounce); PSUM budget:
            # K accum 2 + S^T 2x2 + PV accum 2 = 8 banks exactly.
            outSB = pers.tile([P, NPAIR, QT], F32R, tag="tok1")
            with tc.tile_pool(name="kp", bufs=2) as kp, \
                 tc.tile_pool(name="kq", bufs=2) as kq, \
                 tc.tile_pool(name="atp", bufs=3) as atp, \
                 tc.tile_pool(name="atd", bufs=2) as atd, \
                 tc.tile_pool(name="kps", bufs=1, space="PSUM") as kps, \
                 tc.tile_pool(name="stp", bufs=2, space="PSUM") as stpool, \
                 tc.tile_pool(name="pvp", bufs=2, space="PSUM") as pvpool:
                for pr in range(NPAIR):
                    wk_t = kp.tile([P, DC, P], F32R, tag="w")
                    eng = nc.scalar if pr % 2 else nc.sync
                    eng.dma_start(wk_t[:], wk[pr].rearrange("c p m -> p c m"))
                    kT_sb = kq.tile([P, L], F32R, tag="kts", name=f"kts_{pr}")
                    for half in range(2):
                        pst = [kps.tile([P, 512], F32, tag=f"kmm{i}",
                                        name=f"kps_{pr}_{half}_{i}") for i in range(2)]
                        for dc in range(DC):
                            for i in range(2):
                                t4 = half * 2 + i
                                nc.tensor.matmul(
                                    pst[i], wk_t[:, dc, :],
                                    xT_t[:, dc, t4 * 512:(t4 + 1) * 512],
                                    start=(dc == 0), stop=(dc == DC - 1))
                        for i in range(2):
                            t4 = half * 2 + i
                            nc.vector.tensor_scalar(
                                kT_sb[:, t4 * 512:(t4 + 1) * 512], pst[i],
                                bk_t[:, pr:pr + 1], None, ALU.add)
                    # ---- attention for this pair (kT_sb read in place) ----
                    pvs = [pvpool.tile([P, QT], F32, tag="pv", name=f"pv_{pr}_{h2}")
                           for h2 in range(2)]
                    for gi in range(KT // 2):
                        a = 2 * gi
                        for h2 in range(2):
                            h_idx = 2 * pr + h2
                            rows = slice(h2 * HD, h2 * HD + HD)
                            stp = stpool.tile([P, 1024], F32, tag="st",
                                              name=f"st_{pr}_{a}_{h2}")
                            for j in range(2):
                                kt = a + j
                                nc.tensor.matmul(
                                    stp[:, j * 512:(j + 1) * 512],
                                    kT_sb[rows, kt * P:(kt + 1) * P],
                                    qT_t[rows, pr, :], start=True, stop=True)
                            ptt = atp.tile([P, 2, QT], F32R, tag="pt",
                                           name=f"pt_{pr}_{a}_{h2}")
                            nc.scalar.activation(
                                ptt[:],
                                stp[:].rearrange("p (c n) -> p c n", n=512),
                                AF.Exp, scale=0.125)
                            for j in range(2):
                                kt = a + j
                                vsl = v_aug[:, kt, :].rearrange(
                                    "p (h c) -> p h c", c=HD + 1)[:, h_idx, :]
                                nc.tensor.matmul(pvs[h2][:HD + 1, :], vsl,
                                                 ptt[:, j, :],
                                                 start=(kt == 0), stop=(kt == KT - 1))
                    for h2 in range(2):
                        rows = slice(h2 * HD, h2 * HD + HD)
                        den = atd.tile([1, QT], F32, tag="den", name=f"den_{pr}_{h2}")
                        nc.vector.reciprocal(den[:], pvs[h2][HD:HD + 1, :])
                        denb = atd.tile([HD, QT], F32, tag="denb",
                                        name=f"denb_{pr}_{h2}")
                        nc.gpsimd.partition_broadcast(denb[:], den[:])
                        nc.vector.tensor_tensor(outSB[rows, pr, :], pvs[h2][:HD, :],
                                                denb[:], ALU.mult)

            _att_es.close()
            _ffn_es = ExitStack()
            fp = _ffn_es.enter_context(tc.tile_pool(name="fp", bufs=2))
            fw = _ffn_es.enter_context(tc.tile_pool(name="fw", bufs=1))

            # ================= Output proj + residual + LN1 ===========
            # Two tt-halves with 4 PSUM banks each: LN1 + hT transposes of
            # half A overlap half B's matmuls.
            h_t = pers.tile([P, TT, D], F32, tag="tok2")
            hT_t = pers.tile([P, DC, QT], F32R, tag="tok1")
            with tc.tile_pool(name="op", bufs=1) as op, \
                 tc.tile_pool(name="lnw", bufs=1) as lnw, \
                 tc.tile_pool(name="lnp3", bufs=2) as lnp3, \
                 tc.tile_pool(name="ops", bufs=1, space="PSUM") as ops, \
                 tc.tile_pool(name="tps", bufs=2, space="PSUM") as tps:
                if affine:
                    g1b_t = lnw.tile([P, D], F32, tag="g1b")
                    be1b_t = lnw.tile([P, D], F32, tag="be1b")
                    nc.scalar.dma_start(g1b_t[:], g1b[:])
                    nc.scalar.dma_start(be1b_t[:], be1b[:])
                else:
                    g1b_t = be1b_t = None
                wo_t = op.tile([P, DC, D], F32R, tag="wof")
                for pr in range(NPAIR):
                    eng = nc.scalar if pr % 2 else nc.sync
                    eng.dma_start(wo_t[:, pr, :], wo[pr])
                xq_s = op.tile([P, TT, D], F32, tag="xqs")
                nc.sync.dma_start(xq_s[:], xq.rearrange("(t p) d -> p t d", p=P))
                for half in range(2):
                    tts = (2 * half, 2 * half + 1)
                    pso = [ops.tile([P, 512], F32, tag=f"ao{i}",
                                    name=f"ao_{half}_{i}") for i in range(4)]
                    for pr in range(NPAIR):
                        for i, tt in enumerate(tts):
                            for oc in range(2):
                                nc.tensor.matmul(
                                    pso[i * 2 + oc],
                                    outSB[:, pr, tt * P:(tt + 1) * P],
                                    wo_t[:, pr, oc * 512:(oc + 1) * 512],
                                    start=(pr == 0), stop=(pr == NPAIR - 1))
                    for i, tt in enumerate(tts):
                        for oc in range(2):
                            nc.vector.tensor_tensor(
                                h_t[:, tt, oc * 512:(oc + 1) * 512],
                                pso[i * 2 + oc],
                                xq_s[:, tt, oc * 512:(oc + 1) * 512], ALU.add)
                        _layernorm(nc, lnp3, h_t[:, tt, :], h_t[:, tt, :],
                                   g1b_t, be1b_t, eps_t, zero_t, affine)
                        for dc in range(DC):
                            pst = tps.tile([P, P], F32, tag="tp",
                                           name=f"tp_{tt}_{dc}")
                            nc.tensor.transpose(
                                pst[:], h_t[:, tt, dc * P:(dc + 1) * P], ident[:])
                            nc.vector.tensor_copy(
                                hT_t[:, dc, tt * P:(tt + 1) * P], pst[:])

            # ================= FFN + LN2 ==============================
            with tc.tile_pool(name="ft", bufs=2) as ft, \
                 tc.tile_pool(name="lnp4", bufs=2) as lnp4:
                ff1 = pers.tile([P, FC, QT], F32R, tag="blobA")
                with tc.tile_pool(name="f1s", bufs=4, space="PSUM") as f1s:
                    for fc4 in range(FC // 4):
                        w1_t = fp.tile([P, 4, DC, P], F32R, tag="wstream")
                        eng = nc.scalar if fc4 % 2 else nc.sync
                        eng.dma_start(w1_t[:], w1[4 * fc4:4 * fc4 + 4].rearrange(
                            "f c p m -> p f c m"))
                        for f in range(4):
                            fc = 4 * fc4 + f
                            ps = f1s.tile([P, 512], F32, tag="mm")
                            for dc in range(DC):
                                nc.tensor.matmul(ps[:], w1_t[:, f, dc, :],
                                                 hT_t[:, dc, :],
                                                 start=(dc == 0), stop=(dc == DC - 1))
                            # fused bias + relu
                            nc.vector.tensor_scalar(ff1[:, fc, :], ps[:],
                                                    b1_t[:, fc:fc + 1], 0.0,
                                                    ALU.add, ALU.max)

                b2b_t = fw.tile([P, D], F32, tag="b2b")
                nc.scalar.dma_start(b2b_t[:], b2b[:])
                # fold the fc2 bias into the residual while ff1 runs
                for tt in range(TT):
                    nc.vector.tensor_tensor(h_t[:, tt, :], h_t[:, tt, :],
                                            b2b_t[:], ALU.add)
                if affine:
                    g2b_t = fw.tile([P, D], F32, tag="g2b")
                    be2b_t = fw.tile([P, D], F32, tag="be2b")
                    nc.scalar.dma_start(g2b_t[:], g2b[:])
                    nc.scalar.dma_start(be2b_t[:], be2b[:])
                else:
                    g2b_t = be2b_t = None
                with tc.tile_pool(name="f2s", bufs=1, space="PSUM") as f2s:
                    pss = [f2s.tile([P, 512], F32, tag=f"ff2_{i}", name=f"ff2_{i}") for i in range(8)]
                    for fc4 in range(FC // 4):
                        w2_t = fp.tile([P, 4, D], F32R, tag="wstream")
                        eng = nc.scalar if fc4 % 2 else nc.sync
                        eng.dma_start(w2_t[:], w2[4 * fc4:4 * fc4 + 4].rearrange(
                            "f p m -> p f m"))
                        for f in range(4):
                            fc = 4 * fc4 + f
                            for tt in range(TT):
                                for oc in range(2):
                                    nc.tensor.matmul(
                                        pss[tt * 2 + oc],
                                        ff1[:, fc, tt * P:(tt + 1) * P],
                                        w2_t[:, f, oc * 512:(oc + 1) * 512],
                                        start=(fc == 0), stop=(fc == FC - 1))
                    for tt in range(TT):
                        t2 = ft.tile([P, D], F32, tag="t2")
                        for oc in range(2):
                            nc.vector.tensor_tensor(
                                t2[:, oc * 512:(oc + 1) * 512],
                                pss[tt * 2 + oc],
                                h_t[:, tt, oc * 512:(oc + 1) * 512], ALU.add)
                        _layernorm(nc, lnp4, t2[:], t2[:], g2b_t, be2b_t, eps_t, zero_t, affine)
                        nc.sync.dma_start(
                            y.rearrange("(t p) d -> p t d", p=P)[:, tt, :], t2[:])
            _ffn_es.close()

    nc.compile()
    return nc


def _layernorm(nc, pool, dst, src, g_t, be_t, eps_t, zero_t, affine):
    """dst = (src - mean)/sqrt(var + eps) [* g + be], row-wise over 1024.

    var = E[x^2] - mu^2 (safe here: |mu| << rms). The mean-reduce (DVE) and
    square+sum (ACT, accum_out) run concurrently; one Newton step refines
    rsqrt. c doubles as the square scratch before holding (src - mu).
    """
    mu = pool.tile([P, 1], F32, tag="ln_mu")
    nc.vector.tensor_reduce(mu[:], src, mybir.AxisListType.X, ALU.add)
    nc.vector.tensor_scalar_mul(mu[:], mu[:], 1.0 / D)
    c = pool.tile([P, D], F32, tag="ln_c")
    ss = pool.tile([P, 1], F32, tag="ln_ss")
    nc.scalar.activation(c[:], src, AF.Square, accum_out=ss[:])
    vv = pool.tile([P, 1], F32, tag="ln_v")
    nc.vector.tensor_scalar(vv[:], ss[:], 1.0 / D, EPS, ALU.mult, ALU.add)
    m2 = pool.tile([P, 1], F32, tag="ln_m2")
    nc.vector.tensor_tensor(m2[:], mu[:], mu[:], ALU.mult)
    nc.vector.tensor_tensor(vv[:], vv[:], m2[:], ALU.subtract)
    s = pool.tile([P, 1], F32, tag="ln_s")
    nc.scalar.activation(s[:], vv[:], AF.Sqrt, bias=zero_t[:])
    r = pool.tile([P, 1], F32, tag="ln_r")
    nc.vector.reciprocal(r[:], s[:])
    # one Newton step: r <- r * (1.5 - 0.5 * vv * r^2)
    t = pool.tile([P, 1], F32, tag="ln_t")
    nc.vector.tensor_tensor(t[:], r[:], r[:], ALU.mult)
    nc.vector.tensor_tensor(t[:], t[:], vv[:], ALU.mult)
    nc.vector.tensor_scalar(t[:], t[:], -0.5, 1.5, ALU.mult, ALU.add)
    nc.vector.tensor_tensor(r[:], r[:], t[:], ALU.mult)
    nc.vector.tensor_scalar(c[:], src, mu[:], None, ALU.subtract)
    nc.vector.tensor_scalar(dst, c[:], r[:], None, ALU.mult)
    if affine:
        nc.vector.tensor_tensor(dst, dst, g_t[:], ALU.mult)
        nc.vector.tensor_tensor(dst, dst, be_t[:], ALU.add)


def make_in_maps(x, w_qkv, b_qkv, w_o, b_o, g1, be1, w1, b1, w2, b2, g2, be2):
    f = np.float32
    x = np.asarray(x, f)
    w_qkv = np.asarray(w_qkv, f)
    b_qkv = np.asarray(b_qkv, f)
    bc = lambda v: np.ascontiguousarray(
        np.broadcast_to(np.asarray(v, f).reshape(1, D), (P, D)))
    shared = {
        "wq": np.ascontiguousarray(
            w_qkv[:, :D].reshape(DC, P, NPAIR, P).transpose(2, 0, 1, 3)),
        "wk": np.ascontiguousarray(
            w_qkv[:, D:2 * D].reshape(DC, P, NPAIR, P).transpose(2, 0, 1, 3)),
        "wv": np.ascontiguousarray(
            w_qkv[:, 2 * D:].reshape(DC, P, 2, 512).transpose(2, 0, 1, 3)),
        "wo": np.ascontiguousarray(np.asarray(w_o, f).reshape(DC, P, D)),
        "w1": np.ascontiguousarray(
            np.asarray(w1, f).reshape(DC, P, FC, P).transpose(2, 0, 1, 3)),
        "w2": np.ascontiguousarray(np.asarray(w2, f).reshape(FC, P, D)),
        "bq": np.ascontiguousarray(b_qkv[:D].reshape(NPAIR, P).T),
        "bk": np.ascontiguousarray(b_qkv[D:2 * D].reshape(NPAIR, P).T),
        "b1": np.ascontiguousarray(np.asarray(b1, f).reshape(FC, P).T),
        "bvb": bc(b_qkv[2 * D:]), "b2b": bc(b2),
        "g1b": bc(g1), "be1b": bc(be1), "g2b": bc(g2), "be2b": bc(be2),
        "vones": np.ones((P, KT), f),
    }
    in_maps = []
    for c in range(8):
        n, qi = divmod(c, 4)
        xTn = np.ascontiguousarray(x[n].T)
        m = dict(shared)
        m["xT"] = xTn
        m["xTq"] = np.ascontiguousarray(xTn[:, qi * QT:(qi + 1) * QT])
        m["xq"] = np.ascontiguousarray(x[n, qi * QT:(qi + 1) * QT, :]
                                 + np.asarray(b_o, f).reshape(1, D))
        in_maps.append(m)
    return in_maps


def get_nc(affine=True):
    if affine not in _CACHED_NC:
        _CACHED_NC[affine] = _build_nc(affine)
    return _CACHED_NC[affine]


def kernel(**inputs):
    in_maps = make_in_maps(**inputs)
    affine = not (np.all(np.asarray(inputs["g1"]) == 1)
                  and np.all(np.asarray(inputs["be1"]) == 0)
                  and np.all(np.asarray(inputs["g2"]) == 1)
                  and np.all(np.asarray(inputs["be2"]) == 0))
    nc = get_nc(affine)
    res = run_bass_kernel_spmd(nc, in_maps, list(range(8))).results
    y = np.empty((NB, L, D), np.float32)
    for c in range(8):
        n, qi = divmod(c, 4)
        y[n, qi * QT:(qi + 1) * QT] = res[c]["y"]
    return y


if __name__ == "__main__":
    rng = np.random.default_rng(0)
    demo = {
        "x": rng.standard_normal((NB, L, D)).astype(np.float32),
        "w_qkv": rng.standard_normal((D, 3 * D)).astype(np.float32) * 0.03,
        "b_qkv": rng.standard_normal(3 * D).astype(np.float32) * 0.03,
        "w_o": rng.standard_normal((D, D)).astype(np.float32) * 0.03,
        "b_o": rng.standard_normal(D).astype(np.float32) * 0.03,
        "g1": np.ones(D, np.float32), "be1": np.zeros(D, np.float32),
        "w1": rng.standard_normal((D, FF)).astype(np.float32) * 0.03,
        "b1": rng.standard_normal(FF).astype(np.float32) * 0.03,
        "w2": rng.standard_normal((FF, D)).astype(np.float32) * 0.015,
        "b2": rng.standard_normal(D).astype(np.float32) * 0.015,
        "g2": np.ones(D, np.float32), "be2": np.zeros(D, np.float32),
    }
    out = kernel(**demo)
    print("kernel output:", out.shape, out.dtype, np.abs(out).mean())
